# revision 2
# baseline (speedup 1.0000x reference)
"""AdjacencyMatchingLoss on 8 trn2 NeuronCores — M2-form rewrite.

Math (per batch b):
    A[p,q]  = (d_hw[p,q] == 1)
    G[i,q]  = sum_p P_b[i,p] A[p,q]                  (= P_b A)
    M2[m,i] = sum_q P_b[m,q] G[i,q]                  (= P_b A P_b^T, transposed)
    S_b     = sum_e w_be * M2[dst_be, src_be]
    loss    = -mean_b( S_b / max(sum_e w_be, eps) )

Device pipeline (fp8 DoubleRow matmuls):
    stage 1: Gt[q,i] = sum_p A[p,q] Pt[p,i]          (128 DR matmuls / batch)
    stage 2: M2[m,i] = sum_q Pt[q,m]^T Gt[q,i]       (32 DR matmuls / batch)
    tail:    S_b = sum(W ⊙ M2), sw_b = sum(W)        (one DVE TTR + one reduce)

W is the edge-weight image: the host scatters w_be to
[partition = dst % 128, offset = (dst//128)*512 + src] in fp16 (exact layout
encode of the edge list; duplicate (dst,src) weights summed). M2 is copied
PSUM->SBUF as fp16 scaled by 2^-4 (fits fp16 range); the 16x is folded into
the final scalar tail.

Host ships fp8/fp16 casts + layouts only: A8 [p,q] p-major, Pt8 = P^T
[p-major, i], W16 per batch. Data-parallel over batch: 2 batches/core,
host sums 8 scalars.
"""

import numpy as np

B, NLOG, NPHYS, E = 16, 512, 2048, 2048
NCORES = 8
BLOC = B // NCORES          # batches per core
NI = NLOG // 128            # 4  m/i-chunks per batch
NP = NPHYS // 128           # 16 p/q-chunks
EPS = 1e-8
GT_BIAS = -256.0            # center Gt before fp8: residual ~N(0, 12) keeps
                            # e4m3 quantization unbiased (raw Gt~256 collapses
                            # onto the coarse ulp-8 grid). Tail adds back
                            # 256*<rowP, wrow> with rowP from a cheap ones
                            # matvec on the PE.

_CACHE = {}


def _emit(tc, aps):
    from contextlib import ExitStack

    from concourse import mybir

    nc = tc.nc
    f32 = mybir.dt.float32
    f16 = mybir.dt.float16
    f8 = mybir.dt.float8e4
    AO = mybir.AluOpType
    ACT_COPY = mybir.ActivationFunctionType.Copy
    DRM = mybir.MatmulPerfMode.DoubleRow

    A8_ap = aps["A8"]
    Pt8_ap = aps["Pt8"]
    W16_ap = aps["W16"]
    iota_ap = aps["iota"]      # [128, 512] f16 (PE warmup fodder)
    out_ap = aps["out"]

    ctx = ExitStack()
    with ctx:
        const = ctx.enter_context(tc.tile_pool(name="const", bufs=1))
        # bufs=2: body n+1's A8/Pt DMAs stream in during body n's compute
        big = ctx.enter_context(tc.tile_pool(name="big", bufs=2))
        gtp = ctx.enter_context(tc.tile_pool(name="gtp", bufs=2))
        m2p = ctx.enter_context(tc.tile_pool(name="m2p", bufs=2))
        wim = ctx.enter_context(tc.tile_pool(name="wim", bufs=2))
        swl = ctx.enter_context(tc.tile_pool(name="swl", bufs=4))
        accp = ctx.enter_context(tc.tile_pool(name="accp", bufs=4))
        scr = ctx.enter_context(tc.tile_pool(name="scr", bufs=2))
        # PSUM budget (8 banks): ps1 = 3 x [128,1024] (stage-1 pairs +
        # stage-2 halves, ring-of-3 for copy/matmul overlap); psr = 2 x
        # [128,512] (rowP matvec + scalar tail)
        ps1 = ctx.enter_context(tc.tile_pool(name="ps1", bufs=3, space="PSUM"))
        psr = ctx.enter_context(tc.tile_pool(name="psr", bufs=2, space="PSUM"))

        # loop-invariant constants on the gpsimd software-DGE queue (keeps
        # sync/scalar bulk-DMA counters clean for loop bodies).
        iota16 = const.tile([128, 512], f16)
        nc.gpsimd.dma_start(iota16, iota_ap)
        ones8 = const.tile([128, 2, 128], f8)
        nc.vector.memset(ones8, 1.0)

        def _body():
            # ---- bulk input DMA, partition-major so each transfer is one
            # contiguous run per partition ----
            W16 = wim.tile([128, BLOC, NPHYS], f16, tag="W16")
            nc.scalar.dma_start(W16, W16_ap.rearrange("p (b n) -> p b n", b=BLOC))

            if aps.get("_warm", True) and "_nrep" not in aps:
                # single-shot: keep the PE HAM window busy while inputs stream
                ps_w = ps1.tile([128, 1024], f32, tag="ps1")
                for wi in range(12):
                    nc.tensor.matmul(ps_w[:, 0:512], iota16[:, 0:128],
                                     iota16[:, 0:512])

            A8 = big.tile([128, NP, NPHYS], f8, tag="A8")
            A_src = A8_ap.rearrange("p (k q) -> p k q", k=NP)
            nc.scalar.dma_start(A8[:, 0:NP // 2, :], A_src[:, 0:NP // 2, :])
            nc.sync.dma_start(A8[:, NP // 2:NP, :], A_src[:, NP // 2:NP, :])
            Pt = big.tile([128, BLOC * NP, 512], f8, tag="Pt")
            nc.sync.dma_start(
                Pt, Pt8_ap.rearrange("p (k i) -> p k i", k=BLOC * NP))

            # ---- stage 1 (both batches): Gt[q,i] = sum_p A[p,q] Pt[p,i],
            # stored centered (Gt - 256) in fp8; each group's copy is split
            # ACT/DVE so the last chunk lands fast ----
            gts = [gtp.tile([128, NP, 512], f8, tag="gt8", name=f"Gt8_{b}")
                   for b in range(BLOC)]
            # paired over batches with half-size psum groups (2 banks each,
            # 2 groups in flight) so each A8 stationary load serves both
            # batches while copies still overlap the next group's matmuls
            for qg in range(NP // 2):
                psa = ps1.tile([128, 1024], f32, tag="ps1")
                psb = ps1.tile([128, 1024], f32, tag="ps1")
                pss = [psa, psb]
                for q2 in range(2):
                    qc = qg * 2 + q2
                    for k in range(NP // 2):
                        for b in range(BLOC):
                            nc.tensor.matmul(
                                pss[b][:, q2 * 512:(q2 + 1) * 512],
                                A8[:, 2 * k:2 * k + 2, qc * 128:(qc + 1) * 128],
                                Pt[:, b * NP + 2 * k:b * NP + 2 * k + 2, :],
                                start=(k == 0),
                                stop=(k == NP // 2 - 1),
                                perf_mode=DRM,
                            )
                for b in range(BLOC):
                    eng = nc.scalar if (qg + b) % 2 == 0 else None
                    if eng is not None:
                        nc.scalar.activation(
                            gts[b][:, qg * 2:qg * 2 + 2, :],
                            pss[b].rearrange("p (c i) -> p c i", c=2),
                            ACT_COPY, bias=GT_BIAS,
                        )
                    else:
                        nc.vector.tensor_scalar_add(
                            gts[b][:, qg * 2:qg * 2 + 2, :],
                            pss[b].rearrange("p (c i) -> p c i", c=2),
                            GT_BIAS,
                        )



            # ---- stage 2 (both batches): M2c[m,i] = sum_q P[m,q] Gtc[q,i];
            # rowP[m] = sum_q P[m,q] via ones matvec;
            # tail: S = sum(W⊙M2c) + 256*<rowP, wrow> ----
            accs = []
            for b in range(BLOC):
                # rowP as [1, 512] on partition 0, then DMA-rearranged to
                # [128, 4] (partition-major m) to pair with wrow
                if not aps.get("_no_rowp"):
                    psrt = psr.tile([128, 512], f32, tag="psr")
                    for k in range(NP // 2):
                        nc.tensor.matmul(
                            psrt,
                            ones8,
                            Pt[:, b * NP + 2 * k:b * NP + 2 * k + 2, :],
                            start=(k == 0),
                            stop=(k == NP // 2 - 1),
                            perf_mode=DRM,
                        )
                    rowpl = swl.tile([1, 512], f32, tag="rowpl")
                    nc.scalar.copy(rowpl, psrt[0:1, :])
                    rowp = accp.tile([128, NI], f32, tag="rowp")
                    if not aps.get("_no_bounce"):
                        # partition-redistribute [1,512] -> [128,4] via a DRAM
                        # bounce (SBUF->SBUF partition-crossing DMA corrupts on
                        # HW); both DMAs on one queue so FIFO order serializes
                        # write->read
                        rp_scr = aps["rp_scr"][b]
                        nc.gpsimd.dma_start(rp_scr, rowpl)
                        nc.gpsimd.dma_start(
                            rowp, rp_scr.rearrange("o (c p) -> p (o c)", p=128))
                    else:
                        nc.vector.memset(rowp, 0.0)
                m2sb = m2p.tile([128, NI, 512], f16, tag="m2sb")
                for half in range(2):
                    ps2 = ps1.tile([128, 1024], f32, tag="ps1")
                    for ml in range(2):
                        mc = half * 2 + ml
                        for k in range(NP // 2):
                            nc.tensor.matmul(
                                ps2[:, ml * 512:(ml + 1) * 512],
                                Pt[:, b * NP + 2 * k:b * NP + 2 * k + 2,
                                   mc * 128:(mc + 1) * 128],
                                gts[b][:, 2 * k:2 * k + 2, :],
                                start=(k == 0),
                                stop=(k == NP // 2 - 1),
                                perf_mode=DRM,
                            )
                    if half == 0:
                        nc.scalar.activation(
                            m2sb[:, 0:2, :],
                            ps2.rearrange("p (c i) -> p c i", c=2), ACT_COPY)
                    else:
                        nc.vector.tensor_scalar_add(
                            m2sb[:, 2:4, :],
                            ps2.rearrange("p (c i) -> p c i", c=2), 0.0)
                wrow = swl.tile([128, NI], f32, tag="wrow")
                nc.vector.tensor_reduce(
                    wrow, W16[:, b, :].rearrange("p (c i) -> p c i", c=NI),
                    axis=mybir.AxisListType.X, op=AO.add)
                # sw partials from wrow (cheap [128,4] pass, not a second
                # full W sweep)
                nc.vector.tensor_reduce(acc4[:, 2 + b:3 + b], wrow,
                                        axis=mybir.AxisListType.X, op=AO.add)
                # fp16 TTR wedges the exec unit on this runtime; use
                # TT (DVE) + activation-accumulate (ACT) instead
                tmp = scr.tile([128, 2048], f16, tag="tmp")
                nc.vector.tensor_tensor(
                    tmp, W16[:, b, :], m2sb.rearrange("p c i -> p (c i)"),
                    AO.mult)
                trash = scr.tile([128, 2048], f16, tag="trash")
                pacc = accp.tile([128, 1], f32, tag="pacc")
                nc.scalar.activation(trash, tmp, ACT_COPY, accum_out=pacc)
                tmp4 = swl.tile([128, NI], f32, tag="tmp4")
                nc.vector.tensor_tensor(tmp4, rowp, wrow, AO.mult)
                trash4 = swl.tile([128, NI], f32, tag="trash4")
                pacc2 = accp.tile([128, 1], f32, tag="pacc2")
                nc.scalar.activation(trash4, tmp4, ACT_COPY,
                                     scale=-GT_BIAS, accum_out=pacc2)
                nc.scalar.add(acc4[:, b:b + 1], pacc, pacc2[:, 0:1])

        def _tail():
            if not aps.get("_mm_tail2"):
                # cross-partition reduce on Pool (no PE in the tail)
                red4 = swl.tile([1, 4], f32, tag="red4")
                nc.gpsimd.tensor_reduce(red4, acc4,
                                        axis=mybir.AxisListType.C, op=AO.add)
            else:
                red4t = psr.tile([128, 512], f32, tag="psr")
                for c in range(4):
                    nc.tensor.matmul(red4t[:, c:c + 1], onesf,
                                     acc4[:, c:c + 1])
                red4 = red4t
            rsw2 = swl.tile([1, 2], f32, tag="rsw2")
            nc.vector.reciprocal(rsw2, red4[0:1, 2:4])
            t0 = swl.tile([1, 4], f32, tag="t0")
            nc.scalar.activation(t0[0:1, 2:3], red4[0:1, 0:1],
                                 ACT_COPY, scale=rsw2[0:1, 0:1])
            nc.scalar.activation(t0[0:1, 3:4], red4[0:1, 1:2],
                                 ACT_COPY, scale=rsw2[0:1, 1:2])
            nc.scalar.dma_start(out_ap, t0[0:1, 2:4])

        acc4 = accp.tile([128, 4], f32, tag="acc4")
        onesf = const.tile([128, 128], f32)
        nc.vector.memset(onesf, 1.0)

        if "_nrep" in aps:
            nrt = const.tile([1, 1], mybir.dt.int32)
            nc.sync.dma_start(nrt, aps["_nrep"])
            nval = nc.values_load(nrt[0:1, 0:1], min_val=1, max_val=4096,
                                  skip_runtime_bounds_check=True)
            with tc.For_i(0, nval, 1):
                _body()
                _tail()
        else:
            _body()
            _tail()


def build(repeat=1, loop_rt=False, probe_flags=()):
    import concourse.tile as tile
    from concourse import bacc, mybir

    f32 = mybir.dt.float32
    f16 = mybir.dt.float16
    f8 = mybir.dt.float8e4
    i32 = mybir.dt.int32
    nc = bacc.Bacc(
        "TRN2", target_bir_lowering=False, debug=False, num_devices=NCORES
    )
    aps = {
        "A8": nc.dram_tensor("A8", [128, NP * NPHYS], f8, kind="ExternalInput").ap(),
        "Pt8": nc.dram_tensor("Pt8", [128, BLOC * NP * NLOG], f8, kind="ExternalInput").ap(),
        "W16": nc.dram_tensor("W16", [128, BLOC * NPHYS], f16, kind="ExternalInput").ap(),
        "iota": nc.dram_tensor("iota", [128, 512], f16, kind="ExternalInput").ap(),
        "out": nc.dram_tensor("out", [1, 2], f32, kind="ExternalOutput").ap(),
        "rp_scr": [
            nc.dram_tensor(f"rp_scr{b}", [1, 512], f32, kind="Internal").ap()
            for b in range(BLOC)
        ],
        "ac_scr": nc.dram_tensor("ac_scr", [1, 512], f32, kind="Internal").ap(),
    }
    for fl in probe_flags:
        if isinstance(fl, tuple):
            aps[fl[0]] = fl[1]
        else:
            aps[fl] = True
    if loop_rt:
        aps["_nrep"] = nc.dram_tensor("nrep", [1, 1], i32, kind="ExternalInput").ap()
    with tile.TileContext(nc) as tc:
        for _ in range(repeat):
            _emit(tc, aps)
    nc.compile()
    return nc


def _pmajor(x):
    """[.., k*128, m] -> [128, ..*k*m]: partition-major relayout so device
    DMAs are one contiguous run per partition."""
    lead = x.shape[:-2]
    n, m = x.shape[-2], x.shape[-1]
    k = n // 128
    y = x.reshape(*lead, k, 128, m)
    order = (len(lead) + 1,) + tuple(range(len(lead))) + (len(lead), len(lead) + 2)
    return np.ascontiguousarray(y.transpose(order).reshape(128, -1))


def _w_image(edge_src, edge_dst, edge_w):
    """[B, E] edges -> [B, 128, NPHYS] fp16 weight image:
    W[b, dst%128, (dst//128)*512 + src] = sum of w over duplicate (dst,src)."""
    Bn, En = edge_src.shape
    img = np.zeros((Bn, 128, NPHYS), np.float32)
    part = (edge_dst % 128).astype(np.int64)
    off = (edge_dst // 128).astype(np.int64) * NLOG + edge_src
    for b in range(Bn):
        np.add.at(img[b], (part[b], off[b]), edge_w[b])
    return img.astype(np.float16)


def shard_inputs(P, d_hw, edge_src, edge_dst, edge_w):
    import ml_dtypes

    f8 = ml_dtypes.float8_e4m3fn
    P = np.asarray(P, dtype=np.float32)
    Pt8 = np.ascontiguousarray(P.transpose(0, 2, 1)).astype(f8)
    A8_pm = _pmajor((np.asarray(d_hw) == 1).astype(f8))
    edge_src = np.asarray(edge_src, dtype=np.int64)
    edge_dst = np.asarray(edge_dst, dtype=np.int64)
    edge_w = np.asarray(edge_w, dtype=np.float32)
    Wimg = _w_image(edge_src, edge_dst, edge_w)       # [B, 128, NPHYS] f16
    iota = np.broadcast_to(np.arange(512, dtype=np.float16), (128, 512)).copy()
    in_maps = []
    for c in range(NCORES):
        sl = slice(c * BLOC, (c + 1) * BLOC)
        w_pm = np.ascontiguousarray(
            Wimg[sl].transpose(1, 0, 2).reshape(128, BLOC * NPHYS))
        in_maps.append(
            {
                "A8": A8_pm,
                "Pt8": _pmajor(Pt8[sl]),
                "W16": w_pm,
                "iota": iota,
            }
        )
    return in_maps


def kernel(P, d_hw, edge_src, edge_dst, edge_w):
    from concourse.bass_utils import run_bass_kernel_spmd

    if "nc" not in _CACHE:
        _CACHE["nc"] = build()
    nc = _CACHE["nc"]
    in_maps = shard_inputs(P, d_hw, edge_src, edge_dst, edge_w)
    res = run_bass_kernel_spmd(nc, in_maps, core_ids=list(range(NCORES)))
    partial = sum(float(res.results[c]["out"][0, 0]) +
                  float(res.results[c]["out"][0, 1]) for c in range(NCORES))
    return np.float32(-partial / B)
